# revision 1
# baseline (speedup 1.0000x reference)
"""Trainium2 Bass kernel for nn_Decoder (2-layer LSTM autoregressive decoder).

Model (see reference): B=256 batch, T=512 steps, H=256 hidden, 2 LSTM layers,
scalar (F=1) autoregressive feedback through an output projection, final
output = scalar MSE loss vs `sequence`.

Strategy
--------
- Data-parallel over batch: 8 NeuronCores x 32 batches each. Weights
  replicated. Zero cross-core communication; loss assembled on host.
- Per core, per step, gates are computed as [128, 256] PSUM tiles with
  partition = 32*gate + batch (gate order i,f,o,g) via 4-way column-tiled
  matmuls (M=32 stationary h, weights moving, all bf16).
- The scalar pred feedback is folded into a rank-1 matrix
  Am = 0.5*outer(W_out, W_ih_l0), so gates_l0(t) = H0(t-1)@A0 + H1(t-1)@Am
  (+ effective bias row via a K=1 matmul) and no per-step matvec sits on the
  recurrence critical path.
- One fused tanh activation per layer-step computes tanh for the g gate and
  tanh(x/2) for i,f,o (per-partition scale), using sigmoid(x) = (tanh(x/2)+1)/2.
  States are kept doubled (H=2h bf16, S=2c fp32) so the (y+1) fixups fold into
  three scalar_tensor_tensor ops:
      p = (y_i + 1) * y_g            # = 2*sigmoid(i)*gtilde
      q = (y_f + 1) * S              # = 4*sigmoid(f)*c
      S' = 0.5*q + p                 # = 2*c_new
      tc = tanh(0.5*S')              # = tanh(c_new)
      H' = (y_o + 1) * tc            # = 2*h_new
  The 0.5 for the doubled H is pre-folded into every weight matrix.
- Activated gates are transposed on the tensor engine (2x 128x128) so the
  cell update runs h-major ([128, 2, 32] tiles) and H' lands directly in the
  [K, M] layout the next step's matmuls need as stationary operand.
- h1 history is written straight into a [128, T*64] SBUF buffer by the H'
  update op and DMA'd to HBM in chunks; preds + loss are computed on host.
"""

import sys

import numpy as np

if "/opt/trn_rl_repo" not in sys.path:
    sys.path.insert(0, "/opt/trn_rl_repo")

import ml_dtypes

B, T, H = 256, 512, 256
NCORES = 8
BSH = B // NCORES  # 32 batches per core
NG = 4  # gates
GW = H  # gate width in j-dim (256)

BF16 = ml_dtypes.bfloat16

# device gate order: i, f, o, g (so sigmoid gates are partitions 0..95)
# pytorch row order in the 4H dim: i, f, g, o
_PERM = np.concatenate([
    np.arange(0, 256),       # i
    np.arange(256, 512),     # f
    np.arange(768, 1024),    # o
    np.arange(512, 768),     # g
])

# column offsets inside the big bf16 const block [128, CB_COLS]
_OFF_A0 = 0
_OFF_AM = 2048
_OFF_A1I = 4096
_OFF_A1H = 6144
_OFF_IDENT = 8192
_OFF_H0I = 8320
_OFF_H1I = 8384
_OFF_BIAS0 = 8448     # partition 0 only
_OFF_BIAS0S0 = 9472   # partition 0 only
_OFF_BIAS1 = 10496    # partition 0 only
_OFF_ONES = 11520     # partition 0 only
CB_COLS = 11552

_OFF_WO = 11552       # 0.5*W_out per k-half: [128, 2]
_OFF_PB = 11554       # pred/bias K=2 block rows: p0=W_ih_l0, p1=bias0_eff
_OFF_P2I = 12578      # pred2 init: row0 = 0 (pred slot), row1 = ones
CB_COLS2 = 12610

# f32 const block [128, CF_COLS]: scalev | s0i | s1i
_OFF_SCALEV = 0
_OFF_S0I = 1
_OFF_S1I = 65
CF_COLS = 129


def _to_dev_mat(a):
    """[256 k, 1024 j] fp32 -> [128, 2048] with layout [p, (khalf, j)]."""
    assert a.shape == (256, 1024)
    return a.reshape(2, 128, 1024).transpose(1, 0, 2).reshape(128, 2048)


def _to_dev_state(a):
    """[32 b, 256 k] -> [128, 64] with layout [p, (khalf, b)]."""
    assert a.shape == (BSH, H)
    return a.T.reshape(2, 128, BSH).transpose(1, 0, 2).reshape(128, 2 * BSH)


def _prep_host(inputs):
    """Precompute device const blocks from the raw inputs."""
    W_ih_l0 = np.asarray(inputs["W_ih_l0"], np.float32)
    W_hh_l0 = np.asarray(inputs["W_hh_l0"], np.float32)
    b_ih_l0 = np.asarray(inputs["b_ih_l0"], np.float32)
    b_hh_l0 = np.asarray(inputs["b_hh_l0"], np.float32)
    W_ih_l1 = np.asarray(inputs["W_ih_l1"], np.float32)
    W_hh_l1 = np.asarray(inputs["W_hh_l1"], np.float32)
    b_ih_l1 = np.asarray(inputs["b_ih_l1"], np.float32)
    b_hh_l1 = np.asarray(inputs["b_hh_l1"], np.float32)
    W_out = np.asarray(inputs["W_out"], np.float32)
    b_out = np.asarray(inputs["b_out"], np.float32)
    z = np.asarray(inputs["z"], np.float32)

    # 0.5 scale folds the doubled stored state H=2h into each h-consumer.
    A0 = 0.5 * W_hh_l0.T[:, _PERM]                          # [256, 1024]
    Am = 0.5 * np.outer(W_out[0], W_ih_l0[:, 0])[:, _PERM]  # [256, 1024]
    A1i = 0.5 * W_ih_l1.T[:, _PERM]
    A1h = 0.5 * W_hh_l1.T[:, _PERM]
    bias0 = (b_ih_l0 + b_hh_l0 + b_out[0] * W_ih_l0[:, 0])[_PERM]  # t >= 1
    bias0_s0 = (b_ih_l0 + b_hh_l0)[_PERM]                          # t == 0
    bias1 = (b_ih_l1 + b_hh_l1)[_PERM]

    per_core_cb = []
    per_core_cf = []
    for c in range(NCORES):
        zc = z[c * BSH:(c + 1) * BSH]  # [32, 256]
        zt = _to_dev_state(2.0 * zc)   # [128, 64]

        cb = np.zeros((128, CB_COLS2), np.float32)
        cb[:, _OFF_A0:_OFF_A0 + 2048] = _to_dev_mat(A0)
        cb[:, _OFF_AM:_OFF_AM + 2048] = _to_dev_mat(Am)
        cb[:, _OFF_A1I:_OFF_A1I + 2048] = _to_dev_mat(A1i)
        cb[:, _OFF_A1H:_OFF_A1H + 2048] = _to_dev_mat(A1h)
        cb[:, _OFF_IDENT:_OFF_IDENT + 128] = np.eye(128, dtype=np.float32)
        cb[:, _OFF_H0I:_OFF_H0I + 64] = zt
        cb[:, _OFF_H1I:_OFF_H1I + 64] = zt
        cb[0, _OFF_BIAS0:_OFF_BIAS0 + 1024] = bias0
        cb[0, _OFF_BIAS0S0:_OFF_BIAS0S0 + 1024] = bias0_s0
        cb[0, _OFF_BIAS1:_OFF_BIAS1 + 1024] = bias1
        cb[0, _OFF_ONES:_OFF_ONES + BSH] = 1.0
        # 0.5*W_out arranged [p, khalf] for the predT matvec
        cb[:, _OFF_WO:_OFF_WO + 2] = (0.5 * W_out[0]).reshape(2, 128).T
        # K=2 pred/bias block: row0 = W_ih_l0 (unscaled), row1 = bias0_eff
        cb[0, _OFF_PB:_OFF_PB + 1024] = W_ih_l0[:, 0][_PERM]
        cb[1, _OFF_PB:_OFF_PB + 1024] = bias0
        cb[1, _OFF_P2I:_OFF_P2I + BSH] = 1.0
        per_core_cb.append(cb.astype(BF16))

        cf = np.zeros((128, CF_COLS), np.float32)
        cf[:, _OFF_SCALEV] = 0.5
        cf[96:, _OFF_SCALEV] = 1.0  # g-gate partitions: plain tanh
        cf[:, _OFF_S0I:_OFF_S0I + 64] = zt
        cf[:, _OFF_S1I:_OFF_S1I + 64] = zt
        per_core_cf.append(cf)

    flags = {
        "has_bias0": bool(np.any(bias0 != 0)),
        "has_bias0_s0": bool(np.any(bias0_s0 != 0)),
        "has_bias1": bool(np.any(bias1 != 0)),
        # when the plain l0 bias is zero, bias0_eff == b_out * W_ih_l0 and
        # folds into the pred scalar: (predD + b_out) * W_ih_l0
        "fold_bout": not bool(np.any(bias0_s0 != 0)),
        "b_out": float(b_out[0]),
    }
    return per_core_cb, per_core_cf, flags


def _build_program(t_steps, flags, reps=1):
    import concourse.bass as bass
    import concourse.mybir as mybir
    import concourse.tile as tile
    from concourse import bacc
    from contextlib import ExitStack

    f32 = mybir.dt.float32
    bf = mybir.dt.bfloat16
    ADD = mybir.AluOpType.add
    MULT = mybir.AluOpType.mult
    TANH = mybir.ActivationFunctionType.Tanh

    nc = bacc.Bacc("TRN2", target_bir_lowering=False, debug=False)

    dcb = nc.dram_tensor("cb", [128, CB_COLS2], bf, kind="ExternalInput")
    dcf = nc.dram_tensor("cf", [128, CF_COLS], f32, kind="ExternalInput")
    dhist = nc.dram_tensor("hist", [128, t_steps * 2 * BSH], bf,
                           kind="ExternalOutput")

    with tile.TileContext(nc) as tc, ExitStack() as ctx:
        const = ctx.enter_context(tc.tile_pool(name="const", bufs=1))

        cbt = const.tile([128, CB_COLS2], bf, tag="cb")
        nc.sync.dma_start(cbt[:], dcb[:, :])
        cft = const.tile([128, CF_COLS], f32, tag="cf")
        nc.sync.dma_start(cft[:], dcf[:, :])

        cbv = cbt[:]
        A0v = cbv[:, _OFF_A0:_OFF_A0 + 2048].rearrange("p (c j) -> p c j", c=2)
        Amv = cbv[:, _OFF_AM:_OFF_AM + 2048].rearrange("p (c j) -> p c j", c=2)
        A1iv = cbv[:, _OFF_A1I:_OFF_A1I + 2048].rearrange("p (c j) -> p c j", c=2)
        A1hv = cbv[:, _OFF_A1H:_OFF_A1H + 2048].rearrange("p (c j) -> p c j", c=2)
        ident = cbv[:, _OFF_IDENT:_OFF_IDENT + 128]
        h0iv = cbv[:, _OFF_H0I:_OFF_H0I + 64]
        h1iv = cbv[:, _OFF_H1I:_OFF_H1I + 64].rearrange("p (c b) -> p c b", c=2)
        bias0 = cbv[0:1, _OFF_BIAS0:_OFF_BIAS0 + 1024]
        bias0s0 = cbv[0:1, _OFF_BIAS0S0:_OFF_BIAS0S0 + 1024]
        bias1 = cbv[0:1, _OFF_BIAS1:_OFF_BIAS1 + 1024]
        ones = cbv[0:1, _OFF_ONES:_OFF_ONES + BSH]
        wov = cbv[:, _OFF_WO:_OFF_WO + 2]
        pbv = cbv[:, _OFF_PB:_OFF_PB + 1024]
        scalev = cft[:][:, _OFF_SCALEV:_OFF_SCALEV + 1]
        s0iv = cft[:][:, _OFF_S0I:_OFF_S0I + 64]
        s1iv = cft[:][:, _OFF_S1I:_OFF_S1I + 64]

        # mutable state tiles, initialized by on-device copies
        h0 = const.tile([128, 2 * BSH], bf, tag="h0")
        s0 = const.tile([128, 2 * BSH], f32, tag="s0")
        s1 = const.tile([128, 2 * BSH], f32, tag="s1")
        nc.vector.tensor_copy(h0[:], h0iv)
        nc.vector.tensor_copy(s0[:], s0iv)
        nc.vector.tensor_copy(s1[:], s1iv)

        h0v = h0[:].rearrange("p (c b) -> p c b", c=2)
        s0v = s0[:].rearrange("p (c b) -> p c b", c=2)
        s1v = s1[:].rearrange("p (c b) -> p c b", c=2)

        hist = const.tile([128, t_steps * 2 * BSH], bf, tag="hist")
        histv = hist[:].rearrange("p (t c b) -> p t c b", t=t_steps, c=2)

        pg0 = ctx.enter_context(tc.tile_pool(name="pg0", bufs=2, space="PSUM"))
        pg1 = ctx.enter_context(tc.tile_pool(name="pg1", bufs=2, space="PSUM"))
        pyt = ctx.enter_context(tc.tile_pool(name="pyt", bufs=2, space="PSUM"))
        ypool = ctx.enter_context(tc.tile_pool(name="ypool", bufs=2))
        ytpool = ctx.enter_context(tc.tile_pool(name="ytpool", bufs=2))
        cellp = ctx.enter_context(tc.tile_pool(name="cellp", bufs=2))

        def mm_parts(gps, parts, start_i0=True, stop_last=True):
            n = len(parts)
            for i, (lh, rv, c) in enumerate(parts):
                for g in range(NG):
                    rhs = rv[0:1, g * GW:(g + 1) * GW] if c is None \
                        else rv[:, c, g * GW:(g + 1) * GW]
                    nc.tensor.matmul(
                        gps[32 * g:32 * (g + 1), :],
                        lh if c is None else lh[:, c, :],
                        rhs,
                        start=(start_i0 and i == 0),
                        stop=(stop_last and i == n - 1),
                        tile_position=(0, 32 * g),
                    )

        def cell(layer, gps, s_view, h_out_view):
            """Activation + transpose + h-major LSTM cell update."""
            y = ypool.tile([128, 256], bf, tag=f"y{layer}")
            nc.scalar.activation(y[:], gps[:, :], TANH, scale=scalev)
            tp = pyt.tile([128, 256], bf, tag="tp")
            nc.tensor.transpose(tp[:, 0:128], y[:, 0:128], ident)
            nc.tensor.transpose(tp[:, 128:256], y[:, 128:256], ident)
            yt = ytpool.tile([128, 256], bf, tag=f"yt{layer}")
            nc.vector.tensor_copy(yt[:], tp[:])
            ytv = yt[:].rearrange("p (c q b) -> p c q b", c=2, q=NG)
            yi = ytv[:, :, 0, :]
            yf = ytv[:, :, 1, :]
            yo = ytv[:, :, 2, :]
            yg = ytv[:, :, 3, :]
            p = cellp.tile([128, 2 * BSH], bf, tag=f"p{layer}")
            pv = p[:].rearrange("p (c b) -> p c b", c=2)
            nc.vector.scalar_tensor_tensor(pv, yi, 1.0, yg, ADD, MULT)
            q = cellp.tile([128, 2 * BSH], f32, tag=f"q{layer}")
            qv = q[:].rearrange("p (c b) -> p c b", c=2)
            nc.vector.scalar_tensor_tensor(qv, yf, 1.0, s_view, ADD, MULT)
            nc.vector.scalar_tensor_tensor(s_view, qv, 0.5, pv, MULT, ADD)
            tch = cellp.tile([128, 2 * BSH], bf, tag=f"tc{layer}")
            nc.scalar.activation(tch[:], s_view.rearrange("p c b -> p (c b)"),
                                 TANH, scale=0.5)
            tcv = tch[:].rearrange("p (c b) -> p c b", c=2)
            nc.vector.scalar_tensor_tensor(h_out_view, yo, 1.0, tcv, ADD, MULT)

        # pred staging tile: partition 0 = predT (rewritten each step),
        # partition 1 = constant ones (for the fused bias row).
        pred_sb = const.tile([1, BSH], bf, tag="pred_sb")
        ppred = ctx.enter_context(tc.tile_pool(name="ppred", bufs=2,
                                               space="PSUM"))

        from contextlib import nullcontext
        loop_cm = tc.For_i(0, reps, 1) if reps > 1 else nullcontext()
        with loop_cm:
          for t in range(t_steps):
            h1prev = histv[:, t - 1, :, :] if t > 0 else h1iv

            # pred(t) = h1(t-1) @ W_out as a tiny transposed matvec, then
            # fold pred + effective bias into one K=2 rank-1 matmul group.
            if t > 0:
                pt = ppred.tile([1, BSH], f32, tag="pt")
                for c in range(2):
                    nc.tensor.matmul(
                        pt[:, :], wov[:, c:c + 1], h1prev[:, c, :],
                        start=(c == 0), stop=(c == 1), tile_position=(0, 0),
                    )
                nc.vector.tensor_scalar_add(pred_sb[0:1, :], pt[:, :],
                                            flags["b_out"])

            # layer-0 gates: hh-part + (pred,bias) rank-2 part
            g0 = pg0.tile([128, 256], f32, tag="g0")
            parts = [(h0v, A0v, 0), (h0v, A0v, 1)]
            if t > 0:
                parts.append((pred_sb[0:1, :], pbv, "K1"))
                if not flags["fold_bout"] and flags["has_bias0"]:
                    parts.append((ones, bias0, None))
            elif flags["has_bias0_s0"]:
                parts.append((ones, bias0s0, None))
            n0 = len(parts)
            for i, (lh, rv, c) in enumerate(parts):
                for g in range(NG):
                    if c == "K1":
                        lhs, rhs = lh, rv[0:1, g * GW:(g + 1) * GW]
                    elif c is None:
                        lhs, rhs = lh, rv[0:1, g * GW:(g + 1) * GW]
                    else:
                        lhs, rhs = lh[:, c, :], rv[:, c, g * GW:(g + 1) * GW]
                    nc.tensor.matmul(
                        g0[32 * g:32 * (g + 1), :], lhs, rhs,
                        start=(i == 0), stop=(i == n0 - 1),
                        tile_position=(0, 32 * g),
                    )

            # layer-1 hh-part: emitted after g0 so it fills PE idle time
            # while the layer-0 activation/cell chain runs.
            g1 = pg1.tile([128, 256], f32, tag="g1")
            mm_parts(g1, [(h1prev, A1hv, 0), (h1prev, A1hv, 1)],
                     start_i0=True, stop_last=False)

            cell(0, g0, s0v, h0v)

            # layer-1 ih-part (needs fresh h0) + optional bias, then cell.
            tail = [(h0v, A1iv, 0), (h0v, A1iv, 1)]
            if flags["has_bias1"]:
                tail.append((ones, bias1, None))
            mm_parts(g1, tail, start_i0=False, stop_last=True)

            cell(1, g1, s1v, histv[:, t, :, :])

            if t % 32 == 31 or t == t_steps - 1:
                lo = (t // 32) * 32 * 2 * BSH
                hi = (t + 1) * 2 * BSH
                nc.sync.dma_start(dhist[:, lo:hi], hist[:][:, lo:hi])

    nc.compile()
    return nc


def _postprocess(results, inputs, t_steps):
    W_out = np.asarray(inputs["W_out"], np.float32)
    b_out = np.asarray(inputs["b_out"], np.float32)
    sequence = np.asarray(inputs["sequence"], np.float32)

    h1_all = np.empty((B, t_steps, H), np.float64)
    for c in range(NCORES):
        histd = np.asarray(results[c]["hist"]).astype(np.float32)
        # [128, t*2*32] -> [t, b, khalf, p] -> [t, b, 256]
        h1 = histd.reshape(128, t_steps, 2, BSH).transpose(1, 3, 2, 0)
        h1 = h1.reshape(t_steps, BSH, H) * 0.5  # undo doubling
        h1_all[c * BSH:(c + 1) * BSH] = h1.transpose(1, 0, 2)

    preds = h1_all @ W_out[0].astype(np.float64) + np.float64(b_out[0])  # [B, T]
    diff = sequence[:, :t_steps, 0].astype(np.float64) - preds
    loss = np.mean(diff * diff)
    return np.asarray(loss, dtype=np.float32)


def run(inputs, t_steps=T, trace=False):
    """Build + run on 8 cores; returns (loss, bass_results)."""
    from concourse.bass_utils import run_bass_kernel_spmd

    per_core_cb, per_core_cf, flags = _prep_host(inputs)
    nc = _build_program(t_steps, flags)
    in_maps = [{"cb": per_core_cb[c], "cf": per_core_cf[c]}
               for c in range(NCORES)]
    res = run_bass_kernel_spmd(nc, in_maps, list(range(NCORES)), trace=trace)
    loss = _postprocess(res.results, inputs, t_steps)
    return loss, res


def kernel(**inputs) -> np.ndarray:
    loss, _ = run(inputs, T, trace=False)
    return loss



# revision 15
# speedup vs baseline: 1.2362x; 1.2362x over previous
"""Trainium2 Bass kernel for nn_Decoder (2-layer LSTM autoregressive decoder).

Model (see reference): B=256 batch, T=512 steps, H=256 hidden, 2 LSTM layers,
scalar (F=1) autoregressive feedback through an output projection, final
output = scalar MSE loss vs `sequence`.

Strategy
--------
- Data-parallel over batch: 8 NeuronCores x 32 batches each. Weights
  replicated. Zero cross-core communication; loss assembled on host.
- Per core, per step, gates are computed as [128, 256] PSUM tiles with
  partition = 32*gate + batch (gate order i,f,o,g) via 4-way column-tiled
  matmuls (M=32 stationary h, weights moving, all bf16).
- The scalar pred feedback is folded into a rank-1 matrix
  Am = 0.5*outer(W_out, W_ih_l0), so gates_l0(t) = H0(t-1)@A0 + H1(t-1)@Am
  (+ effective bias row via a K=1 matmul) and no per-step matvec sits on the
  recurrence critical path.
- One fused tanh activation per layer-step computes tanh for the g gate and
  tanh(x/2) for i,f,o (per-partition scale), using sigmoid(x) = (tanh(x/2)+1)/2.
  States are kept doubled (H=2h bf16, S=2c fp32) so the (y+1) fixups fold into
  three scalar_tensor_tensor ops:
      p = (y_i + 1) * y_g            # = 2*sigmoid(i)*gtilde
      q = (y_f + 1) * S              # = 4*sigmoid(f)*c
      S' = 0.5*q + p                 # = 2*c_new
      tc = tanh(0.5*S')              # = tanh(c_new)
      H' = (y_o + 1) * tc            # = 2*h_new
  The 0.5 for the doubled H is pre-folded into every weight matrix.
- Activated gates are transposed on the tensor engine (2x 128x128) so the
  cell update runs h-major ([128, 2, 32] tiles) and H' lands directly in the
  [K, M] layout the next step's matmuls need as stationary operand.
- h1 history is written straight into a [128, T*64] SBUF buffer by the H'
  update op and DMA'd to HBM in chunks; preds + loss are computed on host.
"""

import sys

import numpy as np

if "/opt/trn_rl_repo" not in sys.path:
    sys.path.insert(0, "/opt/trn_rl_repo")

import ml_dtypes

B, T, H = 256, 512, 256
NCORES = 8
BSH = B // NCORES  # 32 batches per core
NG = 4  # gates
GW = H  # gate width in j-dim (256)

BF16 = ml_dtypes.bfloat16

# device gate order: i, f, o, g (so sigmoid gates are partitions 0..95)
# pytorch row order in the 4H dim: i, f, g, o
_PERM = np.concatenate([
    np.arange(0, 256),       # i
    np.arange(256, 512),     # f
    np.arange(768, 1024),    # o
    np.arange(512, 768),     # g
])

# column offsets inside the big bf16 const block [128, CB_COLS]
_OFF_A0 = 0
_OFF_AM = 2048
_OFF_A1I = 4096
_OFF_A1H = 6144
_OFF_IDENT = 8192
_OFF_H0I = 8320
_OFF_H1I = 8384
_OFF_BIAS0 = 8448     # partition 0 only
_OFF_BIAS0S0 = 9472   # partition 0 only
_OFF_BIAS1 = 10496    # partition 0 only
_OFF_ONES = 11520     # partition 0 only
CB_COLS = 11552

_OFF_WO = 11552       # 0.5*W_out per k-half: [128, 2]
_OFF_PB = 11554       # pred/bias K=2 block rows: p0=W_ih_l0, p1=bias0_eff
_OFF_P2I = 12578      # pred2 init: row0 = 0 (pred slot), row1 = ones
CB_COLS2 = 12610

# f32 const block [128, CF_COLS]: scalev | s0i | s1i
_OFF_SCALEV = 0
_OFF_S0I = 1
_OFF_S1I = 65
CF_COLS = 129


# k-row permutation: array row p, k-tile c holds original h-dim pi(p, c).
# Chosen so the DVE 32x32 block-transpose of the batch-major activated
# gates lands h-major state directly in this layout (no PE transpose).
_PI = (64 * (np.arange(128) // 32)[:, None] + 32 * np.arange(2)[None, :]
       + (np.arange(128) % 32)[:, None])  # [128, 2]

# fold the per-gate activation scale (tanh(x/2) for i,f,o; tanh(x) for g)
# into the weight columns, since gates are mixed along the free dim now.
_GATE_SCALE = np.array([0.5, 0.5, 0.5, 1.0], np.float32)


def _to_dev_mat(a):
    """[256 k, 1024 (gate,j)] fp32 -> [128, 2048] = [p, (ktile c, cols)].

    Columns regrouped as (q, gate, jj) with j = 64q + jj so PE col-group q
    computes all four gates for j-quarter q; k-rows permuted by pi.
    """
    assert a.shape == (256, 1024)
    ag = a.reshape(256, 4, 4, 64) * _GATE_SCALE[None, :, None, None]
    ac = ag.transpose(0, 2, 1, 3).reshape(256, 1024)  # [k, (q, gate, jj)]
    return ac[_PI].transpose(0, 1, 2).reshape(128, 2048)  # [p, (c, cols)]


def _to_dev_bias(bias):
    """[1024 (gate,j)] -> [1024 (q, gate, jj)] with gate scale folded."""
    bg = bias.reshape(4, 4, 64) * _GATE_SCALE[:, None, None]
    return bg.transpose(1, 0, 2).reshape(1024)


def _to_dev_state(a):
    """[32 b, 256 k] -> [128, 64] with layout [p, (ktile c, b)], k-permuted."""
    assert a.shape == (BSH, H)
    return a.T[_PI].reshape(128, 2 * BSH)


def _hist_to_h1(histd, t_steps):
    """[128, t*2*32] device hist -> [t, 32, 256] true h1 (undoubled)."""
    tmp = histd.reshape(128, t_steps, 2, BSH).transpose(1, 3, 0, 2)
    tmp = tmp.reshape(t_steps, BSH, 256)  # value order (p, c)
    h1 = np.empty_like(tmp)
    h1[:, :, _PI.reshape(-1)] = tmp
    return h1 * 0.5


def _prep_host(inputs):
    """Precompute device const blocks from the raw inputs."""
    W_ih_l0 = np.asarray(inputs["W_ih_l0"], np.float32)
    W_hh_l0 = np.asarray(inputs["W_hh_l0"], np.float32)
    b_ih_l0 = np.asarray(inputs["b_ih_l0"], np.float32)
    b_hh_l0 = np.asarray(inputs["b_hh_l0"], np.float32)
    W_ih_l1 = np.asarray(inputs["W_ih_l1"], np.float32)
    W_hh_l1 = np.asarray(inputs["W_hh_l1"], np.float32)
    b_ih_l1 = np.asarray(inputs["b_ih_l1"], np.float32)
    b_hh_l1 = np.asarray(inputs["b_hh_l1"], np.float32)
    W_out = np.asarray(inputs["W_out"], np.float32)
    b_out = np.asarray(inputs["b_out"], np.float32)
    z = np.asarray(inputs["z"], np.float32)

    # 0.5 scale folds the doubled stored state H=2h into each h-consumer.
    A0 = 0.5 * W_hh_l0.T[:, _PERM]                          # [256, 1024]
    Am = 0.5 * np.outer(W_out[0], W_ih_l0[:, 0])[:, _PERM]  # [256, 1024]
    A1i = 0.5 * W_ih_l1.T[:, _PERM]
    A1h = 0.5 * W_hh_l1.T[:, _PERM]
    bias0 = (b_ih_l0 + b_hh_l0 + b_out[0] * W_ih_l0[:, 0])[_PERM]  # t >= 1
    bias0_s0 = (b_ih_l0 + b_hh_l0)[_PERM]                          # t == 0
    bias1 = (b_ih_l1 + b_hh_l1)[_PERM]

    per_core_cb = []
    per_core_cf = []
    for c in range(NCORES):
        zc = z[c * BSH:(c + 1) * BSH]  # [32, 256]
        zt = _to_dev_state(2.0 * zc)   # [128, 64]

        cb = np.zeros((128, CB_COLS2), np.float32)
        cb[:, _OFF_A0:_OFF_A0 + 2048] = _to_dev_mat(A0)
        cb[:, _OFF_AM:_OFF_AM + 2048] = _to_dev_mat(Am)
        cb[:, _OFF_A1I:_OFF_A1I + 2048] = _to_dev_mat(A1i)
        cb[:, _OFF_A1H:_OFF_A1H + 2048] = _to_dev_mat(A1h)
        cb[:, _OFF_IDENT:_OFF_IDENT + 128] = np.eye(128, dtype=np.float32)
        cb[:, _OFF_H0I:_OFF_H0I + 64] = zt
        cb[:, _OFF_H1I:_OFF_H1I + 64] = zt
        cb[0, _OFF_BIAS0:_OFF_BIAS0 + 1024] = _to_dev_bias(bias0)
        cb[0, _OFF_BIAS0S0:_OFF_BIAS0S0 + 1024] = _to_dev_bias(bias0_s0)
        cb[0, _OFF_BIAS1:_OFF_BIAS1 + 1024] = _to_dev_bias(bias1)
        cb[0, _OFF_ONES:_OFF_ONES + BSH] = 1.0
        # 0.5*W_out arranged [p, khalf] for the predT matvec
        cb[:, _OFF_WO:_OFF_WO + 2] = (0.5 * W_out[0]).reshape(2, 128).T
        # K=2 pred/bias block: row0 = W_ih_l0 (unscaled), row1 = bias0_eff
        cb[0, _OFF_PB:_OFF_PB + 1024] = W_ih_l0[:, 0][_PERM]
        cb[1, _OFF_PB:_OFF_PB + 1024] = bias0
        cb[1, _OFF_P2I:_OFF_P2I + BSH] = 1.0
        per_core_cb.append(cb.astype(BF16))

        cf = np.zeros((128, CF_COLS), np.float32)
        cf[:, _OFF_SCALEV] = 0.5
        cf[96:, _OFF_SCALEV] = 1.0  # g-gate partitions: plain tanh
        cf[:, _OFF_S0I:_OFF_S0I + 64] = zt
        cf[:, _OFF_S1I:_OFF_S1I + 64] = zt
        per_core_cf.append(cf)

    flags = {
        "has_bias0": bool(np.any(bias0 != 0)),
        "has_bias0_s0": bool(np.any(bias0_s0 != 0)),
        "has_bias1": bool(np.any(bias1 != 0)),
        # when the plain l0 bias is zero, bias0_eff == b_out * W_ih_l0 and
        # folds into the pred scalar: (predD + b_out) * W_ih_l0
        "fold_bout": not bool(np.any(bias0_s0 != 0)),
        "b_out": float(b_out[0]),
    }
    return per_core_cb, per_core_cf, flags


def _build_program(t_steps, flags, reps=1):
    import concourse.bass as bass
    import concourse.mybir as mybir
    import concourse.tile as tile
    from concourse import bacc
    from contextlib import ExitStack

    f32 = mybir.dt.float32
    bf = mybir.dt.bfloat16
    ADD = mybir.AluOpType.add
    MULT = mybir.AluOpType.mult
    TANH = mybir.ActivationFunctionType.Tanh

    nc = bacc.Bacc("TRN2", target_bir_lowering=False, debug=False)

    dcb = nc.dram_tensor("cb", [128, CB_COLS2], bf, kind="ExternalInput")
    dcf = nc.dram_tensor("cf", [128, CF_COLS], f32, kind="ExternalInput")
    dhist = nc.dram_tensor("hist", [128, t_steps * 2 * BSH], bf,
                           kind="ExternalOutput")

    with tile.TileContext(nc) as tc, ExitStack() as ctx:
        const = ctx.enter_context(tc.tile_pool(name="const", bufs=1))

        cbt = const.tile([128, CB_COLS2], bf, tag="cb")
        nc.sync.dma_start(cbt[:], dcb[:, :])
        cft = const.tile([128, CF_COLS], f32, tag="cf")
        nc.sync.dma_start(cft[:], dcf[:, :])

        cbv = cbt[:]
        A0v = cbv[:, _OFF_A0:_OFF_A0 + 2048].rearrange("p (c j) -> p c j", c=2)
        Amv = cbv[:, _OFF_AM:_OFF_AM + 2048].rearrange("p (c j) -> p c j", c=2)
        A1iv = cbv[:, _OFF_A1I:_OFF_A1I + 2048].rearrange("p (c j) -> p c j", c=2)
        A1hv = cbv[:, _OFF_A1H:_OFF_A1H + 2048].rearrange("p (c j) -> p c j", c=2)
        ident = cbv[:, _OFF_IDENT:_OFF_IDENT + 128]
        h0iv = cbv[:, _OFF_H0I:_OFF_H0I + 64]
        h1iv = cbv[:, _OFF_H1I:_OFF_H1I + 64].rearrange("p (c b) -> p c b", c=2)
        bias0 = cbv[0:1, _OFF_BIAS0:_OFF_BIAS0 + 1024]
        bias0s0 = cbv[0:1, _OFF_BIAS0S0:_OFF_BIAS0S0 + 1024]
        bias1 = cbv[0:1, _OFF_BIAS1:_OFF_BIAS1 + 1024]
        ones = cbv[0:1, _OFF_ONES:_OFF_ONES + BSH]
        wov = cbv[:, _OFF_WO:_OFF_WO + 2]
        pbv = cbv[:, _OFF_PB:_OFF_PB + 1024]
        scalev = cft[:][:, _OFF_SCALEV:_OFF_SCALEV + 1]
        s0iv = cft[:][:, _OFF_S0I:_OFF_S0I + 64]
        s1iv = cft[:][:, _OFF_S1I:_OFF_S1I + 64]

        # mutable state tiles, initialized by on-device copies
        # (cell state kept bf16: 2x DVE throughput; loss tolerance is ample)
        h0 = const.tile([128, 2 * BSH], bf, tag="h0")
        s0 = const.tile([128, 2 * BSH], bf, tag="s0")
        s1 = const.tile([128, 2 * BSH], bf, tag="s1")
        nc.vector.tensor_copy(h0[:], h0iv)
        nc.vector.tensor_copy(s0[:], s0iv)
        nc.vector.tensor_copy(s1[:], s1iv)

        h0v = h0[:].rearrange("p (c b) -> p c b", c=2)
        s0v = s0[:].rearrange("p (c b) -> p c b", c=2)
        s1v = s1[:].rearrange("p (c b) -> p c b", c=2)

        hist = const.tile([128, t_steps * 2 * BSH], bf, tag="hist")
        histv = hist[:].rearrange("p (t c b) -> p t c b", t=t_steps, c=2)

        pg0 = ctx.enter_context(tc.tile_pool(name="pg0", bufs=2, space="PSUM"))
        pg1 = ctx.enter_context(tc.tile_pool(name="pg1", bufs=2, space="PSUM"))
        ypool = ctx.enter_context(tc.tile_pool(name="ypool", bufs=2))
        ytpool = ctx.enter_context(tc.tile_pool(name="ytpool", bufs=2))
        cellp = ctx.enter_context(tc.tile_pool(name="cellp", bufs=2))

        def mm_parts(gps, parts, start_i0=True, stop_last=True):
            n = len(parts)
            for i, (lh, rv, c) in enumerate(parts):
                for g in range(NG):
                    rhs = rv[0:1, g * GW:(g + 1) * GW] if c is None \
                        else rv[:, c, g * GW:(g + 1) * GW]
                    nc.tensor.matmul(
                        gps[32 * g:32 * (g + 1), :],
                        lh if c is None else lh[:, c, :],
                        rhs,
                        start=(start_i0 and i == 0),
                        stop=(stop_last and i == n - 1),
                        tile_position=(0, 32 * g),
                    )

        def cell(layer, gps, s_view, h_out_view):
            """Activation + DVE block-transpose + h-major LSTM cell update.

            Gate scale is folded into the weights, so the activation is a
            plain tanh. The 32x32 stream-transpose lands the activated
            gates h-major (in pi-permuted order matching the weights)."""
            y = ypool.tile([128, 256], bf, tag=f"y{layer}")
            nc.scalar.activation(y[:], gps[:, :], TANH)
            yt = ytpool.tile([128, 256], bf, tag=f"yt{layer}")
            nc.vector.transpose(yt[:], y[:])
            ytv = yt[:].rearrange("p (q c b) -> p q c b", c=2, q=NG)
            yi = ytv[:, 0, :, :]
            yf = ytv[:, 1, :, :]
            yo = ytv[:, 2, :, :]
            yg = ytv[:, 3, :, :]
            p = cellp.tile([128, 2 * BSH], bf, tag=f"p{layer}")
            pv = p[:].rearrange("p (c b) -> p c b", c=2)
            nc.vector.scalar_tensor_tensor(pv, yi, 1.0, yg, ADD, MULT)
            q = cellp.tile([128, 2 * BSH], bf, tag=f"q{layer}")
            qv = q[:].rearrange("p (c b) -> p c b", c=2)
            nc.vector.scalar_tensor_tensor(qv, yf, 1.0, s_view, ADD, MULT)
            nc.vector.scalar_tensor_tensor(s_view, qv, 0.5, pv, MULT, ADD)
            tch = cellp.tile([128, 2 * BSH], bf, tag=f"tc{layer}")
            nc.scalar.activation(tch[:], s_view.rearrange("p c b -> p (c b)"),
                                 TANH, scale=0.5)
            tcv = tch[:].rearrange("p (c b) -> p c b", c=2)
            nc.vector.scalar_tensor_tensor(h_out_view, yo, 1.0, tcv, ADD, MULT)

        from contextlib import nullcontext
        loop_cm = tc.For_i(0, reps, 1) if reps > 1 else nullcontext()
        with loop_cm:
          for t in range(t_steps):
            h1prev = histv[:, t - 1, :, :] if t > 0 else h1iv

            # layer-0 gates, pred folded in as the rank-1 matrix Am:
            #   g0 = H0 @ A0 + H1 @ Am + bias0_eff
            # (bias0_eff = b_ih+b_hh+b_out*W_ih; Am = 0.5*outer(W_out, W_ih)).
            # The bias + A0 parts are off the h1 critical edge; Am is last.
            g0 = pg0.tile([128, 256], f32, tag="g0")
            parts = []
            if t > 0:
                if flags["has_bias0"]:
                    parts.append((ones, bias0, None))
                parts += [(h0v, A0v, 0), (h0v, A0v, 1),
                          (h1prev, Amv, 0), (h1prev, Amv, 1)]
            else:
                parts += [(h0v, A0v, 0), (h0v, A0v, 1)]
                if flags["has_bias0_s0"]:
                    parts.append((ones, bias0s0, None))
            n0 = len(parts)
            for i, (lh, rv, c) in enumerate(parts):
                for g in range(NG):
                    if c is None:
                        lhs, rhs = lh, rv[0:1, g * GW:(g + 1) * GW]
                    else:
                        lhs, rhs = lh[:, c, :], rv[:, c, g * GW:(g + 1) * GW]
                    nc.tensor.matmul(
                        g0[32 * g:32 * (g + 1), :], lhs, rhs,
                        start=(i == 0), stop=(i == n0 - 1),
                        tile_position=(0, 32 * g),
                    )

            # layer-1 hh-part: emitted after g0 so it fills PE idle time
            # while the layer-0 activation/cell chain runs.
            g1 = pg1.tile([128, 256], f32, tag="g1")
            mm_parts(g1, [(h1prev, A1hv, 0), (h1prev, A1hv, 1)],
                     start_i0=True, stop_last=False)

            cell(0, g0, s0v, h0v)

            # layer-1 ih-part (needs fresh h0) + optional bias, then cell.
            tail = [(h0v, A1iv, 0), (h0v, A1iv, 1)]
            if flags["has_bias1"]:
                tail.append((ones, bias1, None))
            mm_parts(g1, tail, start_i0=False, stop_last=True)

            cell(1, g1, s1v, histv[:, t, :, :])

            if t % 32 == 31 or t == t_steps - 1:
                lo = (t // 32) * 32 * 2 * BSH
                hi = (t + 1) * 2 * BSH
                nc.sync.dma_start(dhist[:, lo:hi], hist[:][:, lo:hi])

    nc.compile()
    return nc


def _postprocess(results, inputs, t_steps):
    W_out = np.asarray(inputs["W_out"], np.float32)
    b_out = np.asarray(inputs["b_out"], np.float32)
    sequence = np.asarray(inputs["sequence"], np.float32)

    h1_all = np.empty((B, t_steps, H), np.float64)
    for c in range(NCORES):
        histd = np.asarray(results[c]["hist"]).astype(np.float32)
        h1 = _hist_to_h1(histd, t_steps)  # [t, b, 256], un-permuted
        h1_all[c * BSH:(c + 1) * BSH] = h1.transpose(1, 0, 2)

    preds = h1_all @ W_out[0].astype(np.float64) + np.float64(b_out[0])  # [B, T]
    diff = sequence[:, :t_steps, 0].astype(np.float64) - preds
    loss = np.mean(diff * diff)
    return np.asarray(loss, dtype=np.float32)


def run(inputs, t_steps=T, trace=False):
    """Build + run on 8 cores; returns (loss, bass_results)."""
    from concourse.bass_utils import run_bass_kernel_spmd

    per_core_cb, per_core_cf, flags = _prep_host(inputs)
    nc = _build_program(t_steps, flags)
    in_maps = [{"cb": per_core_cb[c], "cf": per_core_cf[c]}
               for c in range(NCORES)]
    res = run_bass_kernel_spmd(nc, in_maps, list(range(NCORES)), trace=trace)
    loss = _postprocess(res.results, inputs, t_steps)
    return loss, res


def kernel(**inputs) -> np.ndarray:
    loss, _ = run(inputs, T, trace=False)
    return loss

